# revision 18
# baseline (speedup 1.0000x reference)
"""CantorSetAttention Trainium2 kernel (8 NeuronCores, data-parallel).

Reference computes, for depths d=0..7, attention of every query against the
tiny Cantor index set S_d (|S_d| = 2,3,5,9,17,33,65,129; sets are nested),
then blends the 8 outputs with w = softmax(scale_weights / scale_temperature).

Fusion used here:
  A[q,j] = sum_d w_d * 1[j in S_d] * E[q,j] / Z_d(q),  E = exp(q.k_j / sqrt(D))
  rows of A sum to exactly 1 (each softmax sums to 1, sum_d w_d = 1), so with
  j* = index 0 (member of every S_d):
     out[q] = sum_{j != j*} A[q,j] * (V[j] - V[j*])  +  V[j*]
  The union minus j* is exactly 128 indices -> fits the 128-partition PE.

Device layout (per core: one batch b = core//2, query rows half = core%2):
  ST[k,q]   = K_128 @ Q^T  (8 fp16 matmuls per 512-query block, f32 PSUM)
  E = exp(ST/32)           (one ScalarE activation per block)
  Z[8,q]    = M^T E + e*   (mask matmul; e* = exp(q.k_{j*}/32) host-supplied)
  R = 1/Z                  (VectorE reciprocal, fp16)
  C[128,q]  = (w*M) R      (weighted-mask matmul)
  A = E * C                (VectorE)
  P[q,:]    = A^T-weighted (V - v*)  (fp16 AV matmuls, K=128)
Host adds v* back and upcasts to f32.
"""

import math

import numpy as np

import concourse.bass as bass
import concourse.mybir as mybir
from concourse.bass_utils import run_bass_kernel_spmd
from concourse.tile import TileContext

B, L, D = 4, 4096, 1024
NCORES = 8
ROWS_PER_CORE = (B * L) // NCORES  # 2048
N_DEPTHS = 8
INV_SQRT_D = 1.0 / math.sqrt(D)
BLK = 512  # query block per ST/E/Z/C round
NBLK = ROWS_PER_CORE // BLK  # 4
F16 = mybir.dt.float16
F32 = mybir.dt.float32


def _cantor_indices(seq_len: int, depth: int) -> np.ndarray:
    pos = [0.0, 1.0]
    for _ in range(depth):
        new = []
        for i in range(len(pos) - 1):
            l, r = pos[i], pos[i + 1]
            new.append(l)
            new.append(l + (r - l) / 3.0)
        new.append(pos[-1])
        pos = new
    p32 = np.asarray(pos, dtype=np.float32)
    idx = (p32 * np.float32(seq_len - 1)).astype(np.int64)
    return np.unique(idx)


def _index_sets():
    sets = [_cantor_indices(L, d) for d in range(N_DEPTHS)]
    union = sets[-1]
    assert union[0] == 0 and len(union) == 129
    cols = union[union != 0]  # 128 non-j* indices, sorted
    member = np.zeros((N_DEPTHS, len(cols)), dtype=np.float32)
    for d, s in enumerate(sets):
        member[d] = np.isin(cols, s)
    return cols, member


_COLS, _MEMBER = _index_sets()

_NC_CACHE = None

_SPILL_SEQ = [0]


def _legalize_sync_commands(nc):
    """Walrus codegen caps sync commands (waits + updates) per ISA
    instruction at 2. Tile's vector-clock sem assignment freely attaches up
    to ~5 waits. Spill excess waits onto standalone EventSemaphore
    instructions inserted just before the offender on the same engine: the
    engine queue stalls there first, so semantics are identical."""
    import concourse.mybir as mybir

    for f in nc.m.functions:
        for bb in f.blocks:
            insts = bb.instructions
            idx = 0
            while idx < len(insts):
                inst = insts[idx]
                si = inst.sync_info
                if si is None:
                    idx += 1
                    continue
                waits = list(si.on_wait or [])
                updates = list(si.on_update or [])
                assert len(updates) <= 2, (inst.name, updates)
                # Drain lowers to the tiny CTRL_NO struct: one sync slot only.
                cap = 1 if isinstance(inst, mybir.InstDrain) else 2
                keep = max(0, cap - len(updates))
                if len(waits) <= keep:
                    idx += 1
                    continue
                spill, keep_waits = waits[: len(waits) - keep], waits[
                    len(waits) - keep :
                ]
                inst.sync_info = mybir.SyncInfo(
                    on_wait=keep_waits, on_update=updates
                )
                pos = idx
                for i in range(0, len(spill), 2):
                    _SPILL_SEQ[0] += 1
                    ev = mybir.InstEventSemaphore(
                        name=f"WSPILL-{_SPILL_SEQ[0]}", ins=[], outs=[]
                    )
                    ev.engine = inst.engine
                    ev.sync_info = mybir.SyncInfo(
                        on_wait=spill[i : i + 2], on_update=[]
                    )
                    insts.insert(pos, ev)
                    pos += 1
                    idx += 1
                idx += 1


def _build_nc(nrep=1, mode="full"):
    # mode: "full" | "dma" (skip compute) | "compute" (skip per-rep DMAs)
    nc = bass.Bass()
    qt = nc.declare_dram_parameter("qt", [D, ROWS_PER_CORE], F16, isOutput=False)
    kt = nc.declare_dram_parameter("kt", [D, 128], F16, isOutput=False)
    vp = nc.declare_dram_parameter("vp", [128, D], F16, isOutput=False)
    mt = nc.declare_dram_parameter("mt", [128, N_DEPTHS], F16, isOutput=False)
    m8w = nc.declare_dram_parameter("m8w", [N_DEPTHS, 128], F16, isOutput=False)
    ones8 = nc.declare_dram_parameter("ones8", [1, N_DEPTHS], F16, isOutput=False)
    est = nc.declare_dram_parameter("est", [1, ROWS_PER_CORE], F16, isOutput=False)
    out = nc.declare_dram_parameter("out", [ROWS_PER_CORE, D], F16, isOutput=True)

    with TileContext(nc) as tc:
        with (
            tc.tile_pool(name="const", bufs=1) as cpool,
            tc.tile_pool(name="qts", bufs=1) as qpool,
            tc.tile_pool(name="work", bufs=3) as wpool,
            tc.tile_pool(name="osb", bufs=4) as opool,
            tc.tile_pool(name="ps_a", bufs=2, space="PSUM") as ps_a,
            tc.tile_pool(name="ps_z", bufs=2, space="PSUM") as ps_z,
            tc.tile_pool(name="ps_o", bufs=2, space="PSUM") as ps_o,
        ):
            # --- constants ---
            # kt DRAM [1024, 128] -> one SBUF tile [128, 8, 128]:
            # partition p, chunk c holds DRAM row c*128+p
            kt_all = cpool.tile([128, 8, 128], F16, tag="kt")
            nc.sync.dma_start(
                out=kt_all, in_=kt.rearrange("(c p) j -> p c j", p=128)
            )
            kt_t = [kt_all[:, c, :] for c in range(8)]
            vp_t = cpool.tile([128, D], F16, tag="vp")
            nc.sync.dma_start(out=vp_t, in_=vp[:])
            mt_t = cpool.tile([128, N_DEPTHS], F16, tag="mt")
            nc.sync.dma_start(out=mt_t, in_=mt[:])
            m8w_t = cpool.tile([N_DEPTHS, 128], F16, tag="m8w")
            nc.sync.dma_start(out=m8w_t, in_=m8w[:])
            ones8_t = cpool.tile([1, N_DEPTHS], F16, tag="ones8")
            nc.sync.dma_start(out=ones8_t, in_=ones8[:])
            est_t = cpool.tile([1, ROWS_PER_CORE], F16, tag="est")
            nc.sync.dma_start(out=est_t, in_=est[:])

            qt_r = qt.rearrange("(c p) q -> p c q", p=128)
            q_cache = {}
            for blk in range(NBLK * nrep):
                rep, blk = blk // NBLK, blk % NBLK
                qs = blk * BLK
                # Q^T block: one DMA for all 8 contraction chunks.
                # Distinct tiles per blk: input DMAs carry no WAR waits.
                if mode == "compute" and blk in q_cache:
                    q_b = q_cache[blk]
                else:
                    q_b = qpool.tile([128, 8, BLK], F16, tag=f"qt_{blk}")
                    nc.sync.dma_start(out=q_b, in_=qt_r[:, :, qs : qs + BLK])
                    q_cache[blk] = q_b
                if mode == "dma":
                    # store straight back: isolates DMA in/out path
                    for t in range(BLK // 128):
                        row = blk * BLK + t * 128
                        o_sb = opool.tile([128, D], F16, tag="osb")
                        nc.vector.tensor_copy(o_sb[:, 0:512], q_b[:, 0, 0:512])
                        nc.vector.tensor_copy(o_sb[:, 512:1024], q_b[:, 1, 0:512])
                        nc.sync.dma_start(out=out[row : row + 128, :], in_=o_sb)
                    continue

                st = ps_a.tile([128, BLK], F32, tag="stct")
                for c in range(8):
                    nc.tensor.matmul(
                        st,
                        lhsT=kt_t[c],
                        rhs=q_b[:, c, :],
                        start=(c == 0),
                        stop=(c == 7),
                    )

                et = wpool.tile([128, BLK], F16, tag="et")
                nc.scalar.activation(
                    et, st, mybir.ActivationFunctionType.Exp, scale=float(INV_SQRT_D)
                )

                zt = ps_z.tile([N_DEPTHS, BLK], F32, tag="zt")
                nc.tensor.matmul(zt, lhsT=mt_t, rhs=et, start=True, stop=False)
                nc.tensor.matmul(
                    zt,
                    lhsT=ones8_t,
                    rhs=est_t[:, qs : qs + BLK],
                    start=False,
                    stop=True,
                )

                rt = wpool.tile([N_DEPTHS, BLK], F16, tag="rt")
                with nc.allow_low_precision(reason="attention probs fp16"):
                    nc.vector.reciprocal(rt, zt)

                ct = ps_a.tile([128, BLK], F32, tag="stct")
                nc.tensor.matmul(ct, lhsT=m8w_t, rhs=rt, start=True, stop=True)

                at = wpool.tile([128, BLK], F16, tag="at")
                nc.vector.tensor_mul(at, et, ct)

                for t in range(BLK // 128):
                    row = blk * BLK + t * 128
                    o_ps = ps_o.tile([128, D], F32, tag="ops")
                    nc.tensor.matmul(
                        o_ps[:, 0:512],
                        lhsT=at[:, t * 128 : (t + 1) * 128],
                        rhs=vp_t[:, 0:512],
                        start=True,
                        stop=True,
                    )
                    nc.tensor.matmul(
                        o_ps[:, 512:1024],
                        lhsT=at[:, t * 128 : (t + 1) * 128],
                        rhs=vp_t[:, 512:1024],
                        start=True,
                        stop=True,
                    )
                    o_sb = opool.tile([128, D], F16, tag="osb")
                    with nc.allow_low_precision(reason="fp16 output"):
                        nc.scalar.copy(o_sb[:, 0:512], o_ps[:, 0:512])
                        nc.vector.tensor_copy(o_sb[:, 512:1024], o_ps[:, 512:1024])
                    if mode != "compute" or rep == nrep - 1:
                        nc.sync.dma_start(out=out[row : row + 128, :], in_=o_sb)
    _legalize_sync_commands(nc)
    return nc


def _prepare_in_maps(query, key, value, scale_weights, scale_temperature):
    sw = np.asarray(scale_weights, dtype=np.float64)[:N_DEPTHS]
    temp = float(np.asarray(scale_temperature, dtype=np.float64))
    e = np.exp(sw / temp - np.max(sw / temp))
    w = (e / e.sum()).astype(np.float32)  # [8]

    mt = _MEMBER.T.astype(np.float16)  # [128, 8]
    m8w = (_MEMBER * w[:, None]).astype(np.float16)  # [8, 128]
    ones8 = np.ones((1, N_DEPTHS), dtype=np.float16)

    in_maps = []
    vstars = []
    for core in range(NCORES):
        b, half = core // 2, core % 2
        rows = slice(half * ROWS_PER_CORE, (half + 1) * ROWS_PER_CORE)
        q = np.ascontiguousarray(query[b, rows])  # [2048, D] f32
        k_u = key[b, _COLS]  # [128, D] f32
        vstar = value[b, 0].astype(np.float32)  # [D]
        vp = (value[b, _COLS] - vstar[None, :]).astype(np.float16)
        s0 = q @ key[b, 0]  # [2048] f32
        est = np.exp(s0 * INV_SQRT_D).astype(np.float16)[None, :]  # [1, 2048]
        in_maps.append(
            {
                "qt": np.ascontiguousarray(q.T).astype(np.float16),  # [D, 2048]
                "kt": np.ascontiguousarray(k_u.T).astype(np.float16),  # [D, 128]
                "vp": vp,
                "mt": mt,
                "m8w": m8w,
                "ones8": ones8,
                "est": est,
            }
        )
        vstars.append(vstar)
    return in_maps, vstars


def _run(query, key, value, t, scale_weights, scale_temperature, trace=False):
    global _NC_CACHE
    query = np.asarray(query, dtype=np.float32)
    key = np.asarray(key, dtype=np.float32)
    value = np.asarray(value, dtype=np.float32)
    assert query.shape == (B, L, D)

    in_maps, vstars = _prepare_in_maps(
        query, key, value, scale_weights, scale_temperature
    )
    if _NC_CACHE is None:
        _NC_CACHE = _build_nc()
    res = run_bass_kernel_spmd(
        _NC_CACHE, in_maps, core_ids=list(range(NCORES)), trace=trace
    )

    out = np.empty((B, L, D), dtype=np.float32)
    for core in range(NCORES):
        b, half = core // 2, core % 2
        rows = slice(half * ROWS_PER_CORE, (half + 1) * ROWS_PER_CORE)
        out[b, rows] = res.results[core]["out"].astype(np.float32) + vstars[core][
            None, :
        ]
    return out, res


def kernel(query, key, value, t, scale_weights, scale_temperature):
    out, _ = _run(query, key, value, t, scale_weights, scale_temperature, trace=False)
    return out


# revision 27
# speedup vs baseline: 55.1995x; 55.1995x over previous
"""CantorSetAttention Trainium2 kernel (8 NeuronCores, data-parallel).

Reference computes, for depths d=0..7, attention of every query against the
tiny Cantor index set S_d (|S_d| = 2,3,5,9,17,33,65,129; sets are nested),
then blends the 8 outputs with w = softmax(scale_weights / scale_temperature).

Fusion used here:
  A[q,j] = sum_d w_d * 1[j in S_d] * E[q,j] / Z_d(q),  E = exp(q.k_j / sqrt(D))
  rows of A sum to exactly 1 (each softmax sums to 1, sum_d w_d = 1), so with
  j* = index 0 (member of every S_d):
     out[q] = sum_{j != j*} A[q,j] * (V[j] - V[j*])  +  V[j*]
  The union minus j* is exactly 128 indices -> fits the 128-partition PE.

Device layout (per core: one batch b = core//2, query rows half = core%2):
  ST[k,q]   = K_128 @ Q^T  (8 fp16 matmuls per 512-query block, f32 PSUM)
  E = exp(ST/32)           (one ScalarE activation per block)
  Z[8,q]    = M^T E + e*   (mask matmul; e* = exp(q.k_{j*}/32) host-supplied)
  R = 1/Z                  (VectorE reciprocal, fp16)
  C[128,q]  = (w*M) R      (weighted-mask matmul)
  A = E * C                (VectorE)
  P[q,:]    = A^T-weighted (V - v*)  (fp16 AV matmuls, K=128)
Host adds v* back and upcasts to f32.
"""

import math

import numpy as np

import concourse.bass as bass
import concourse.mybir as mybir
from concourse.bass_utils import run_bass_kernel_spmd
from concourse.tile import TileContext

B, L, D = 4, 4096, 1024
NCORES = 8
ROWS_PER_CORE = (B * L) // NCORES  # 2048
N_DEPTHS = 8
INV_SQRT_D = 1.0 / math.sqrt(D)
BLK = 512  # query block per ST/E/Z/C round
NBLK = ROWS_PER_CORE // BLK  # 4
F16 = mybir.dt.float16
F32 = mybir.dt.float32


def _cantor_indices(seq_len: int, depth: int) -> np.ndarray:
    pos = [0.0, 1.0]
    for _ in range(depth):
        new = []
        for i in range(len(pos) - 1):
            l, r = pos[i], pos[i + 1]
            new.append(l)
            new.append(l + (r - l) / 3.0)
        new.append(pos[-1])
        pos = new
    p32 = np.asarray(pos, dtype=np.float32)
    idx = (p32 * np.float32(seq_len - 1)).astype(np.int64)
    return np.unique(idx)


def _index_sets():
    sets = [_cantor_indices(L, d) for d in range(N_DEPTHS)]
    union = sets[-1]
    assert union[0] == 0 and len(union) == 129
    cols = union[union != 0]  # 128 non-j* indices, sorted
    member = np.zeros((N_DEPTHS, len(cols)), dtype=np.float32)
    for d, s in enumerate(sets):
        member[d] = np.isin(cols, s)
    return cols, member


_COLS, _MEMBER = _index_sets()

_NC_CACHE = None

_SPILL_SEQ = [0]


def _legalize_sync_commands(nc):
    """Walrus codegen caps sync commands (waits + updates) per ISA
    instruction at 2. Tile's vector-clock sem assignment freely attaches up
    to ~5 waits. Spill excess waits onto standalone EventSemaphore
    instructions inserted just before the offender on the same engine: the
    engine queue stalls there first, so semantics are identical."""
    import concourse.mybir as mybir

    for f in nc.m.functions:
        for bb in f.blocks:
            insts = bb.instructions
            idx = 0
            while idx < len(insts):
                inst = insts[idx]
                si = inst.sync_info
                if si is None:
                    idx += 1
                    continue
                waits = list(si.on_wait or [])
                updates = list(si.on_update or [])
                assert len(updates) <= 2, (inst.name, updates)
                # Drain lowers to the tiny CTRL_NO struct: one sync slot only.
                cap = 1 if isinstance(inst, mybir.InstDrain) else 2
                keep = max(0, cap - len(updates))
                if len(waits) <= keep:
                    idx += 1
                    continue
                spill, keep_waits = waits[: len(waits) - keep], waits[
                    len(waits) - keep :
                ]
                inst.sync_info = mybir.SyncInfo(
                    on_wait=keep_waits, on_update=updates
                )
                pos = idx
                for i in range(0, len(spill), 2):
                    _SPILL_SEQ[0] += 1
                    ev = mybir.InstEventSemaphore(
                        name=f"WSPILL-{_SPILL_SEQ[0]}", ins=[], outs=[]
                    )
                    ev.engine = inst.engine
                    ev.sync_info = mybir.SyncInfo(
                        on_wait=spill[i : i + 2], on_update=[]
                    )
                    insts.insert(pos, ev)
                    pos += 1
                    idx += 1
                idx += 1


def _build_nc(nrep=1, mode="full"):
    # mode: "full" | "dma" (skip compute) | "compute" (skip per-rep DMAs)
    nc = bass.Bass()
    qt = nc.declare_dram_parameter("qt", [D, ROWS_PER_CORE], F16, isOutput=False)
    kt = nc.declare_dram_parameter("kt", [D, 128], F16, isOutput=False)
    vp = nc.declare_dram_parameter("vp", [128, D], F16, isOutput=False)
    mt = nc.declare_dram_parameter("mt", [128, N_DEPTHS], F16, isOutput=False)
    m8w = nc.declare_dram_parameter("m8w", [N_DEPTHS, 128], F16, isOutput=False)
    ones8 = nc.declare_dram_parameter("ones8", [1, N_DEPTHS], F16, isOutput=False)
    est = nc.declare_dram_parameter("est", [1, ROWS_PER_CORE], F16, isOutput=False)
    out = nc.declare_dram_parameter("out", [ROWS_PER_CORE, D], F16, isOutput=True)

    with TileContext(nc) as tc:
        with (
            tc.tile_pool(name="const", bufs=1) as cpool,
            tc.tile_pool(name="qts", bufs=1) as qpool,
            tc.tile_pool(name="work", bufs=3) as wpool,
            tc.tile_pool(name="osb", bufs=4) as opool,
            tc.tile_pool(name="ps_a", bufs=3, space="PSUM") as ps_a,
            tc.tile_pool(name="ps_z", bufs=1, space="PSUM") as ps_z,
            tc.tile_pool(name="ps_o", bufs=2, space="PSUM") as ps_o,
        ):
            # --- constants (kt + first Q block first: critical path) ---
            # kt DRAM [1024, 128] -> one SBUF tile [128, 8, 128]:
            # partition p, chunk c holds DRAM row c*128+p
            kt_all = cpool.tile([128, 8, 128], F16, tag="kt")
            kt_r = kt.rearrange("(c p) j -> p c j", p=128)
            qt_r = qt.rearrange("(c p) q -> p c q", p=128)
            q_cache = {}

            def load_q(blk, interleave_kt=False):
                q_b = qpool.tile([128, 8, BLK], F16, tag=f"qt_{blk}")
                s = blk * BLK
                # two DMAs: engine-level parallelism + finer matmul gating
                if interleave_kt:
                    nc.sync.dma_start(out=kt_all[:, 0:4], in_=kt_r[:, 0:4])
                nc.sync.dma_start(
                    out=q_b[:, 0:4, :], in_=qt_r[:, 0:4, s : s + BLK]
                )
                if interleave_kt:
                    nc.sync.dma_start(out=kt_all[:, 4:8], in_=kt_r[:, 4:8])
                nc.sync.dma_start(
                    out=q_b[:, 4:8, :], in_=qt_r[:, 4:8, s : s + BLK]
                )
                q_cache[blk] = q_b
                return q_b

            kt_t = [kt_all[:, c, :] for c in range(8)]
            load_q(0, interleave_kt=True)

            mt_t = cpool.tile([128, N_DEPTHS], F16, tag="mt")
            nc.sync.dma_start(out=mt_t, in_=mt[:])
            ones8_t = cpool.tile([1, N_DEPTHS], F16, tag="ones8")
            nc.sync.dma_start(out=ones8_t, in_=ones8[:])
            est_t = cpool.tile([1, ROWS_PER_CORE], F16, tag="est")
            nc.sync.dma_start(out=est_t, in_=est[:])
            m8w_t = cpool.tile([N_DEPTHS, 128], F16, tag="m8w")
            nc.sync.dma_start(out=m8w_t, in_=m8w[:])
            vp_t = cpool.tile([128, D], F16, tag="vp")
            nc.sync.dma_start(out=vp_t, in_=vp[:])

            def stageZ(rep, blk, et):
                """Z-matmuls + reciprocal: emitted before the NEXT block's ST
                so DVE's recip latency hides under that ST."""
                qs = blk * BLK
                zt = ps_z.tile([N_DEPTHS, BLK], F32, tag="zt")
                nc.tensor.matmul(zt, lhsT=mt_t, rhs=et, start=True, stop=False)
                nc.tensor.matmul(
                    zt,
                    lhsT=ones8_t,
                    rhs=est_t[:, qs : qs + BLK],
                    start=False,
                    stop=True,
                )
                rt = wpool.tile([N_DEPTHS, BLK], F16, tag="rt")
                with nc.allow_low_precision(reason="attention probs fp16"):
                    nc.vector.reciprocal(rt, zt)
                return rt

            def stage1(rep, blk):
                """ST matmuls + exp for a block."""
                if rep == 0 and blk == 0:
                    q_b = q_cache[0]
                elif mode == "compute" and blk in q_cache:
                    q_b = q_cache[blk]
                else:
                    q_b = load_q(blk)

                st = ps_a.tile([128, BLK], F32, tag="stct")
                for c in range(8):
                    nc.tensor.matmul(
                        st,
                        lhsT=kt_t[c],
                        rhs=q_b[:, c, :],
                        start=(c == 0),
                        stop=(c == 7),
                    )
                et = wpool.tile([128, BLK], F16, tag="et")
                nc.scalar.activation(
                    et, st, mybir.ActivationFunctionType.Exp, scale=float(INV_SQRT_D)
                )
                return et

            def stage2(rep, blk, et, rt):
                """C / A / AV / output drain for a block."""
                ct = ps_a.tile([128, BLK], F32, tag="stct")
                nc.tensor.matmul(ct, lhsT=m8w_t, rhs=rt, start=True, stop=True)

                at = wpool.tile([128, BLK], F16, tag="at")
                o_blk = opool.tile([128, BLK // 128, D], F16, tag="osb")
                out_r = out.rearrange("(b t p) d -> p b t d", p=128, t=4)
                for t in range(BLK // 128):
                    sl = slice(t * 128, (t + 1) * 128)
                    # per-tile A = E*C so AV(t) starts as soon as slice t is up
                    nc.vector.tensor_mul(at[:, sl], et[:, sl], ct[:, sl])
                    o_ps = ps_o.tile([128, D], F32, tag="ops")
                    nc.tensor.matmul(
                        o_ps[:, 0:512],
                        lhsT=at[:, sl],
                        rhs=vp_t[:, 0:512],
                        start=True,
                        stop=True,
                    )
                    nc.tensor.matmul(
                        o_ps[:, 512:1024],
                        lhsT=at[:, sl],
                        rhs=vp_t[:, 512:1024],
                        start=True,
                        stop=True,
                    )
                    with nc.allow_low_precision(reason="fp16 output"):
                        nc.scalar.copy(o_blk[:, t, 0:640], o_ps[:, 0:640])
                        nc.vector.tensor_copy(o_blk[:, t, 640:1024], o_ps[:, 640:1024])
                    if mode != "compute" or rep == nrep - 1:
                        # outputs on GPSIMD's SWDGE queue: their waits on the
                        # copy producers must not stall SP's input prefetch
                        nc.gpsimd.dma_start(
                            out=out_r[:, blk, t], in_=o_blk[:, t]
                        )

            if mode == "dma":
                for i in range(NBLK * nrep):
                    rep, blk = i // NBLK, i % NBLK
                    if rep == 0 and blk == 0:
                        q_b = q_cache[0]
                    else:
                        q_b = load_q(blk)
                    o_blk = opool.tile([128, NBLK, D], F16, tag="osb")
                    for t in range(BLK // 128):
                        nc.vector.tensor_copy(o_blk[:, t, 0:512], q_b[:, 0, 0:512])
                        nc.vector.tensor_copy(o_blk[:, t, 512:1024], q_b[:, 1, 0:512])
                    nc.sync.dma_start(
                        out=out.rearrange("(b t p) d -> p b t d", p=128, t=4)[
                            :, blk
                        ],
                        in_=o_blk,
                    )
            else:
                # software pipeline: PE order per step is
                #   Z(i-1), ST(i), C/AV(i-1)
                # so exp(i-1) hides under AV(i-2) and recip(i-1) under ST(i)
                pend = None
                for i in range(NBLK * nrep):
                    rep, blk = i // NBLK, i % NBLK
                    if pend is not None:
                        rt = stageZ(*pend)
                    et = stage1(rep, blk)
                    if pend is not None:
                        stage2(*pend, rt)
                    pend = (rep, blk, et)
                rt = stageZ(*pend)
                stage2(*pend, rt)
    _legalize_sync_commands(nc)
    return nc


def _prepare_in_maps(query, key, value, scale_weights, scale_temperature):
    sw = np.asarray(scale_weights, dtype=np.float64)[:N_DEPTHS]
    temp = float(np.asarray(scale_temperature, dtype=np.float64))
    e = np.exp(sw / temp - np.max(sw / temp))
    w = (e / e.sum()).astype(np.float32)  # [8]

    mt = _MEMBER.T.astype(np.float16)  # [128, 8]
    m8w = (_MEMBER * w[:, None]).astype(np.float16)  # [8, 128]
    ones8 = np.ones((1, N_DEPTHS), dtype=np.float16)

    in_maps = []
    vstars = []
    for core in range(NCORES):
        b, half = core // 2, core % 2
        rows = slice(half * ROWS_PER_CORE, (half + 1) * ROWS_PER_CORE)
        q = np.ascontiguousarray(query[b, rows])  # [2048, D] f32
        k_u = key[b, _COLS]  # [128, D] f32
        vstar = value[b, 0].astype(np.float32)  # [D]
        vp = (value[b, _COLS] - vstar[None, :]).astype(np.float16)
        s0 = q @ key[b, 0]  # [2048] f32
        est = np.exp(s0 * INV_SQRT_D).astype(np.float16)[None, :]  # [1, 2048]
        in_maps.append(
            {
                "qt": np.ascontiguousarray(q.T).astype(np.float16),  # [D, 2048]
                "kt": np.ascontiguousarray(k_u.T).astype(np.float16),  # [D, 128]
                "vp": vp,
                "mt": mt,
                "m8w": m8w,
                "ones8": ones8,
                "est": est,
            }
        )
        vstars.append(vstar)
    return in_maps, vstars


def _run(query, key, value, t, scale_weights, scale_temperature, trace=False):
    global _NC_CACHE
    query = np.asarray(query, dtype=np.float32)
    key = np.asarray(key, dtype=np.float32)
    value = np.asarray(value, dtype=np.float32)
    assert query.shape == (B, L, D)

    in_maps, vstars = _prepare_in_maps(
        query, key, value, scale_weights, scale_temperature
    )
    if _NC_CACHE is None:
        _NC_CACHE = _build_nc()
    res = run_bass_kernel_spmd(
        _NC_CACHE, in_maps, core_ids=list(range(NCORES)), trace=trace
    )

    out = np.empty((B, L, D), dtype=np.float32)
    for core in range(NCORES):
        b, half = core // 2, core % 2
        rows = slice(half * ROWS_PER_CORE, (half + 1) * ROWS_PER_CORE)
        out[b, rows] = res.results[core]["out"].astype(np.float32) + vstars[core][
            None, :
        ]
    return out, res


def kernel(query, key, value, t, scale_weights, scale_temperature):
    out, _ = _run(query, key, value, t, scale_weights, scale_temperature, trace=False)
    return out
